# revision 17
# baseline (speedup 1.0000x reference)
"""Trainium2 Bass kernel for nn_ECODQN_layer (GNN message passing).

Reference computation:
    msgs    = edge_attr[:,None] * x[col]                  # gather by source
    x_agg   = scatter_mean(msgs, row, N)                  # segment mean by dest
    m       = relu(concat([x_agg, x_agg_emb]) @ W_msg + b_msg)
    out     = relu(concat([x, m]) @ W_upd + b_upd)

Strategy (8 NeuronCores, SPMD, no collectives):
  * Host relabels nodes into 8 cores x NWIN windows x 128 slots, DEGREE-
    STRATIFIED (all slots of a window have nearly equal in-degree), so the
    scatter can use fixed diagonal tiles: round r of a window holds each
    dest's r-th edge at partition = dest slot.  The segment-sum is then just
    psum += Identity^T @ (attr * gathered_rows) -- a constant stationary
    matmul, no per-tile one-hot build on DVE at all.
  * Scatter-mean is folded into edge weights on the host
    (attr2 = attr / max(deg,1)).
  * Gathers use GPSIMD dma_gather (int16 indices -> two source tables A/B,
    <=32767 rows each).  ~15k high-demand sources are duplicated into BOTH
    tables so each dest's edges can be split across A/B rounds near-evenly
    (minimizes rounds = descriptors).  4 SWDGE queues parallelize the Q7
    descriptor generation.
  * x rows are stored as bf16 (hi | lo residual) pairs: matmuls run in bf16
    at full throughput while x itself stays exact to ~2^-16; only edge_attr
    is rounded to bf16 (~4e-4 relative output error).  attr scaling runs on
    the Scalar (ACT) engine (its own SBUF ports - no contention with the
    GPSIMD descriptor generator).
  * MLPs per window in f32: H tiles have computed halves (x_aggT / mT via
    PE-transpose) on partitions 0-63 and DMA-preloaded x_agg_emb^T / x^T on
    partitions 64-127; node-major matmuls against replicated weights.
  * Output [128, NWIN*64] per core is unpermuted on the host.
"""

import sys

import numpy as np

if "/opt/trn_rl_repo" not in sys.path:
    sys.path.insert(0, "/opt/trn_rl_repo")

import concourse.bass as bass
import concourse.tile as tile
from concourse import bacc, mybir
from concourse.bass_utils import run_bass_kernel_spmd
from concourse.masks import make_identity

P = 128
D = 64
C = 8

F32 = mybir.dt.float32
BF16 = mybir.dt.bfloat16
I16 = mybir.dt.int16

_PROGRAM_CACHE = {}
LAST_RESULTS = None


# --------------------------------------------------------------------------
# host prep
# --------------------------------------------------------------------------

def _host_prep_diag(x, edge_index, edge_attr, x_agg_emb, W_msg, W_upd, rng_seed=0):
    import ml_dtypes

    N = x.shape[0]
    E = edge_index.shape[1]
    NWIN = int(np.ceil(N / (C * P)))
    rng = np.random.default_rng(rng_seed)

    col = np.ascontiguousarray(edge_index[0]).astype(np.int64)
    row = np.ascontiguousarray(edge_index[1]).astype(np.int64)
    deg = np.bincount(row, minlength=N)
    attr2 = (np.asarray(edge_attr, np.float64) / np.maximum(deg, 1)[row]).astype(
        np.float32
    )

    # degree-stratified node placement: rank r -> window r//1024,
    # core (r%1024)%8, slot (r%1024)//8
    order = np.argsort(-deg, kind="stable")
    rank = np.empty(N, np.int64)
    rank[order] = np.arange(N)
    node_win = rank // (C * P)
    node_core = (rank % (C * P)) % C
    node_slot = (rank % (C * P)) // C

    # --- source table coloring: A/B + dups so each dest splits evenly ---
    DUPCAP = 32767 * 2 - N - 400  # dup slots, with per-table margin
    src_order = np.argsort(col, kind="stable")
    src_starts = np.searchsorted(col[src_order], np.arange(N + 1))
    color = (rng.random(N) < 0.5).astype(np.int8)
    lo_cnt = np.bincount(row, weights=(color[col] == 0), minlength=N).astype(np.int64)
    for _sweep in range(6):
        flips = 0
        for u in range(N):
            a, b = src_starts[u], src_starts[u + 1]
            if a == b:
                continue
            dests = row[src_order[a:b]]
            cur = np.abs(2 * lo_cnt[dests] - deg[dests]).sum()
            delta = -1 if color[u] == 0 else 1
            new = np.abs(2 * (lo_cnt[dests] + delta) - deg[dests]).sum()
            if new < cur:
                color[u] = 1 - color[u]
                np.add.at(lo_cnt, dests, delta)
                flips += 1
        if flips < 100:
            break
    dup = np.zeros(N, bool)
    for _it in range(6):
        fixedm = ~dup[col]
        lo_fixed = np.bincount(row, weights=(fixedm & (color[col] == 0)), minlength=N)
        flex = deg - np.bincount(row, weights=fixedm, minlength=N)
        need = np.clip((deg + 1) // 2 - lo_fixed, 0, flex)
        imb = np.abs(2 * (lo_fixed + need) - deg)
        bad = np.where(imb > 1)[0]
        cap = int(DUPCAP - dup.sum())
        if len(bad) == 0 or cap <= 0:
            break
        # pick dup candidates from the worst dests first
        sev = imb[bad]
        bad = bad[np.argsort(-sev, kind="stable")]
        edge_bad = np.isin(row, bad) & ~dup[col]
        erows = row[edge_bad]
        ecols = col[edge_bad]
        sel = np.argsort(-imb[erows], kind="stable")
        cand = ecols[sel]
        _, first = np.unique(cand, return_index=True)
        cand = cand[np.sort(first)]
        dup[cand[:cap]] = True

    # enforce per-table capacity: nA_pure + ndup <= 32767 (and same for B)
    ndup0 = int(dup.sum())
    for side in (0, 1):
        pure = np.where((~dup) & (color == side))[0]
        excess = len(pure) + ndup0 - 32767
        if excess > 0:
            # flip the pure sources with the lowest flip penalty
            pen = np.zeros(len(pure))
            for i, u in enumerate(pure):
                a, b = src_starts[u], src_starts[u + 1]
                dests = row[src_order[a:b]]
                delta = -1 if side == 0 else 1
                pen[i] = (
                    np.abs(2 * (lo_cnt[dests] + delta) - deg[dests]).sum()
                    - np.abs(2 * lo_cnt[dests] - deg[dests]).sum()
                )
            fl = pure[np.argpartition(pen, excess)[:excess]]
            color[fl] = 1 - side
            for u in fl:
                a, b = src_starts[u], src_starts[u + 1]
                np.add.at(lo_cnt, row[src_order[a:b]], -1 if side == 0 else 1)

    # per-edge table assignment: fixed edges follow their source color;
    # flexible (dup-source) edges fill each dest toward ceil(deg/2) in A.
    fixedm = ~dup[col]
    ecolor = np.where(fixedm, color[col], -1).astype(np.int8)
    lo_fixed = np.bincount(row, weights=(ecolor == 0), minlength=N).astype(np.int64)
    need = np.clip((deg + 1) // 2 - lo_fixed, 0, None)
    fidx = np.where(ecolor == -1)[0]
    forder = fidx[np.argsort(row[fidx], kind="stable")]
    fcnt = np.bincount(row[forder], minlength=N)
    fstarts = np.zeros(N + 1, np.int64)
    fstarts[1:] = np.cumsum(fcnt)
    jwf = np.arange(len(forder)) - fstarts[row[forder]]
    ecolor[forder] = np.where(jwf < need[row[forder]], 0, 1).astype(np.int8)

    # table row ids
    isA = (~dup) & (color == 0)
    isB = (~dup) & (color == 1)
    rowA = np.full(N, -1, np.int64)
    rowB = np.full(N, -1, np.int64)
    nA_pure = int(isA.sum())
    nB_pure = int(isB.sum())
    ndup = int(dup.sum())
    rowA[isA] = np.arange(nA_pure)
    rowA[dup] = nA_pure + np.arange(ndup)
    rowB[isB] = np.arange(nB_pure)
    rowB[dup] = nB_pure + np.arange(ndup)
    NTA = nA_pure + ndup
    NTB = nB_pure + ndup
    assert NTA <= 32767 and NTB <= 32767, (NTA, NTB)

    # bf16 hi|lo pair tables
    xf = np.asarray(x, np.float32)
    xb_hi = xf.astype(ml_dtypes.bfloat16)
    xb_lo = (xf - xb_hi.astype(np.float32)).astype(ml_dtypes.bfloat16)
    xpair = np.concatenate([xb_hi, xb_lo], axis=1)  # [N, 128] bf16
    tabA = np.zeros((NTA, 2 * D), ml_dtypes.bfloat16)
    tabB = np.zeros((NTB, 2 * D), ml_dtypes.bfloat16)
    selA = np.where(rowA >= 0)[0]
    tabA[rowA[selA]] = xpair[selA]
    selB = np.where(rowB >= 0)[0]
    tabB[rowB[selB]] = xpair[selB]

    # --- per-(core,window,table) round counts and the shared schedule ---
    ew = node_win[row]
    ecore = node_core[row]
    eslot = node_slot[row]
    eh = ecolor.astype(np.int64)  # 0=A, 1=B
    key = row * 2 + eh
    korder = np.argsort(key, kind="stable")
    kcnt = np.bincount(key, minlength=2 * N)
    kstarts = np.zeros(2 * N + 1, np.int64)
    kstarts[1:] = np.cumsum(kcnt)
    jw = np.empty(E, np.int64)
    jw[korder] = np.arange(E) - kstarts[key[korder]]

    cntA = kcnt[0::2]
    cntB = kcnt[1::2]
    T_A = np.zeros(NWIN, np.int64)
    T_B = np.zeros(NWIN, np.int64)
    for w in range(NWIN):
        nodes = order[w * C * P : (w + 1) * C * P]
        if len(nodes):
            T_A[w] = cntA[nodes].max()
            T_B[w] = cntB[nodes].max()
    TT = T_A + T_B
    cb = np.zeros(NWIN + 1, np.int64)
    cb[1:] = np.cumsum(TT)
    NTILES = int(cb[-1])

    # per-edge tile column + gather idx position
    etile = cb[ew] + np.where(eh == 0, 0, T_A[ew]) + jw
    attr_arr = np.zeros((C, P, NTILES), np.float32)
    attr_arr[ecore, eslot, etile] = attr2
    idx16 = np.zeros((C, 16, NTILES * 8), np.int16)
    erowid = np.where(eh == 0, rowA[col], rowB[col])
    assert (erowid >= 0).all()
    ipos = jw * P + eslot  # position within the (w, table) gather block
    icol = (etile - jw) * 8 + ipos // 16  # block base col*8 + ipos//16
    idx16[ecore, ipos % 16, icol] = erowid.astype(np.int16)
    idx16 = np.ascontiguousarray(np.tile(idx16, (1, 8, 1)))

    # node tensors in slot order
    node_pos = node_win * P + node_slot
    xs = np.zeros((C, NWIN * P, D), np.float32)
    es = np.zeros((C, NWIN * P, D), np.float32)
    xs[node_core, node_pos] = xf
    es[node_core, node_pos] = np.asarray(x_agg_emb, np.float32)
    xT = np.ascontiguousarray(xs.transpose(0, 2, 1))
    xaeT = np.ascontiguousarray(es.transpose(0, 2, 1))

    W_msg2 = np.ascontiguousarray(W_msg).astype(np.float32)
    W_upd2 = np.ascontiguousarray(
        np.concatenate([W_upd[D:], W_upd[:D]], axis=0)
    ).astype(np.float32)

    meta = dict(
        NWIN=NWIN, T_A=tuple(int(t) for t in T_A), T_B=tuple(int(t) for t in T_B),
        NTA=NTA, NTB=NTB, NTILES=NTILES,
        node_core=node_core, node_pos=node_pos, N=N,
    )
    arrays = dict(
        tabA=np.ascontiguousarray(tabA),
        tabB=np.ascontiguousarray(tabB),
        idx16=idx16,
        attrA=attr_arr,
        xT=xT,
        xaeT=xaeT,
        W_msg2=W_msg2,
        W_upd2=W_upd2,
    )
    return meta, arrays


# --------------------------------------------------------------------------
# program builder
# --------------------------------------------------------------------------

def _build_program_diag(NTA, NTB, NWIN, T_A, T_B, NTILES, with_bias):
    nc = bacc.Bacc(
        "TRN2", target_bir_lowering=False, debug=False, num_devices=C,
        num_swdge_queues=4,
    )

    tabA = nc.dram_tensor("tabA", [NTA, 2 * D], BF16, kind="ExternalInput")
    tabB = nc.dram_tensor("tabB", [NTB, 2 * D], BF16, kind="ExternalInput")
    gidx = nc.dram_tensor("gidx", [P, NTILES * 8], I16, kind="ExternalInput")
    attrA = nc.dram_tensor("attrA", [P, NTILES], F32, kind="ExternalInput")
    xT = nc.dram_tensor("xT", [D, NWIN * P], F32, kind="ExternalInput")
    xaeT = nc.dram_tensor("xaeT", [D, NWIN * P], F32, kind="ExternalInput")
    wm = nc.dram_tensor("wm", [2 * D, D], F32, kind="ExternalInput")
    wu = nc.dram_tensor("wu", [2 * D, D], F32, kind="ExternalInput")
    if with_bias:
        bm = nc.dram_tensor("bm", [P, D], F32, kind="ExternalInput")
        bu = nc.dram_tensor("bu", [P, D], F32, kind="ExternalInput")
    out = nc.dram_tensor("out", [P, NWIN * D], F32, kind="ExternalOutput")

    cb = [0]
    for w in range(NWIN):
        cb.append(cb[-1] + T_A[w] + T_B[w])

    with tile.TileContext(nc) as tc:
        with (
            tc.tile_pool(name="const", bufs=1) as cpool,
            tc.tile_pool(name="gather", bufs=6) as gpool,
            tc.tile_pool(name="gs", bufs=6) as gspool,
            tc.tile_pool(name="small", bufs=3) as spool,
            tc.tile_pool(name="ps_agg", bufs=2, space="PSUM") as ps_agg_pool,
            tc.tile_pool(name="ps_tp", bufs=2, space="PSUM") as ps_tp_pool,
            tc.tile_pool(name="ps_mlp", bufs=2, space="PSUM") as ps_mlp_pool,
        ):
            sb_gidx = cpool.tile([P, NTILES * 8], I16)
            sb_attr = cpool.tile([P, NTILES], F32)
            sb_identf = cpool.tile([P, P], F32)
            sb_identb = cpool.tile([P, P], BF16)
            sb_wm = cpool.tile([2 * D, D], F32)
            sb_wu = cpool.tile([2 * D, D], F32)
            sb_H1 = cpool.tile([P, NWIN * P], F32)
            sb_H2 = cpool.tile([P, NWIN * P], F32)
            sb_out = cpool.tile([P, NWIN * D], F32)
            if with_bias:
                sb_bm = cpool.tile([P, D], F32)
                sb_bu = cpool.tile([P, D], F32)

            head = min(cb[2] * 8 if NWIN > 2 else NTILES * 8, NTILES * 8)
            nc.sync.dma_start(out=sb_gidx[:, :head], in_=gidx[:, :head])
            if head < NTILES * 8:
                nc.sync.dma_start(out=sb_gidx[:, head:], in_=gidx[:, head:])
            nc.sync.dma_start(out=sb_attr[:], in_=attrA[:, :])
            nc.sync.dma_start(out=sb_wm[:], in_=wm[:, :])
            nc.sync.dma_start(out=sb_wu[:], in_=wu[:, :])
            # preloaded halves on partitions 64-127 (computed halves -> 0-63)
            nc.sync.dma_start(out=sb_H1[D : 2 * D, :], in_=xaeT[:, :])
            nc.sync.dma_start(out=sb_H2[D : 2 * D, :], in_=xT[:, :])
            if with_bias:
                nc.sync.dma_start(out=sb_bm[:], in_=bm[:, :])
                nc.sync.dma_start(out=sb_bu[:], in_=bu[:, :])
            make_identity(nc, sb_identf[:])
            nc.vector.tensor_copy(out=sb_identb[:], in_=sb_identf[:])

            for w in range(NWIN):
                TA, TB = T_A[w], T_B[w]
                TT = TA + TB
                if TT == 0:
                    continue
                G = gpool.tile([P, TT * 2 * D], BF16, tag="G")
                ioff = cb[w] * 8
                if TA:
                    nc.gpsimd.dma_gather(
                        out_ap=G[:, : TA * 2 * D].rearrange(
                            "p (t d) -> p t d", d=2 * D
                        ),
                        in_ap=tabA[:, :],
                        idxs_ap=sb_gidx[:, ioff : ioff + TA * 8],
                        num_idxs=TA * P,
                        num_idxs_reg=TA * P,
                        elem_size=2 * D,
                        single_packet=False,
                        queue_num=(2 * w) % 4,
                    )
                if TB:
                    nc.gpsimd.dma_gather(
                        out_ap=G[:, TA * 2 * D :].rearrange(
                            "p (t d) -> p t d", d=2 * D
                        ),
                        in_ap=tabB[:, :],
                        idxs_ap=sb_gidx[:, ioff + TA * 8 : ioff + TT * 8],
                        num_idxs=TB * P,
                        num_idxs_reg=TB * P,
                        elem_size=2 * D,
                        single_packet=False,
                        queue_num=(2 * w + 1) % 4,
                    )
                ps_agg = ps_agg_pool.tile([P, 2 * D], F32)
                for g in range(0, TT, 8):
                    k = min(8, TT - g)
                    Gs = gspool.tile([P, 8 * 2 * D], BF16, tag="gs")
                    a0 = cb[w] + g
                    # scale k rounds in one op: attr broadcast along the
                    # 128-wide round via a stride-0 AP dim
                    nc.any.tensor_tensor(
                        out=Gs[:, : k * 2 * D].rearrange(
                            "p (t d) -> p t d", d=2 * D
                        ),
                        in0=G[
                            :, g * 2 * D : (g + k) * 2 * D
                        ].rearrange("p (t d) -> p t d", d=2 * D),
                        in1=sb_attr[:, a0 : a0 + k].to_broadcast([P, k, 2 * D]),
                        op=mybir.AluOpType.mult,
                    )
                    for j in range(k):
                        t = g + j
                        nc.tensor.matmul(
                            out=ps_agg[:],
                            lhsT=sb_identb[:],
                            rhs=Gs[:, j * 2 * D : (j + 1) * 2 * D],
                            start=(t == 0),
                            stop=(t == TT - 1),
                        )
                # x_agg = hi-part + lo-part (value split)
                xagg = spool.tile([P, D], F32)
                nc.any.tensor_copy(out=xagg[:], in_=ps_agg[:, :D])
                nc.any.tensor_add(out=xagg[:], in0=xagg[:], in1=ps_agg[:, D:])
                ptA = ps_tp_pool.tile([D, P], F32, tag="tp")
                nc.tensor.transpose(ptA[:], xagg[:], sb_identf[:])
                nc.any.tensor_copy(out=sb_H1[0:D, bass.ts(w, P)], in_=ptA[:])
                ps_m = ps_mlp_pool.tile([P, D], F32, tag="mlp")
                nc.tensor.matmul(
                    out=ps_m[:],
                    lhsT=sb_H1[:, bass.ts(w, P)],
                    rhs=sb_wm[:],
                    start=True,
                    stop=True,
                )
                m_sb = spool.tile([P, D], F32)
                if with_bias:
                    nc.any.tensor_add(out=m_sb[:], in0=ps_m[:], in1=sb_bm[:])
                    nc.any.tensor_scalar_max(out=m_sb[:], in0=m_sb[:], scalar1=0.0)
                else:
                    nc.any.tensor_scalar_max(out=m_sb[:], in0=ps_m[:], scalar1=0.0)
                ptB = ps_tp_pool.tile([D, P], F32, tag="tp")
                nc.tensor.transpose(ptB[:], m_sb[:], sb_identf[:])
                nc.any.tensor_copy(out=sb_H2[0:D, bass.ts(w, P)], in_=ptB[:])
                ps_o = ps_mlp_pool.tile([P, D], F32, tag="mlp")
                nc.tensor.matmul(
                    out=ps_o[:],
                    lhsT=sb_H2[:, bass.ts(w, P)],
                    rhs=sb_wu[:],
                    start=True,
                    stop=True,
                )
                if with_bias:
                    o_sb = spool.tile([P, D], F32)
                    nc.any.tensor_add(out=o_sb[:], in0=ps_o[:], in1=sb_bu[:])
                    nc.any.tensor_scalar_max(
                        out=sb_out[:, bass.ts(w, D)], in0=o_sb[:], scalar1=0.0
                    )
                else:
                    nc.any.tensor_scalar_max(
                        out=sb_out[:, bass.ts(w, D)], in0=ps_o[:], scalar1=0.0
                    )

            nc.sync.dma_start(out=out[:, :], in_=sb_out[:])

    nc.finalize()
    return nc


# --------------------------------------------------------------------------
# kernel entry
# --------------------------------------------------------------------------

def kernel(x, edge_index, edge_attr, x_agg_emb, W_msg, b_msg, W_upd, b_upd):
    x = np.asarray(x, np.float32)
    x_agg_emb = np.asarray(x_agg_emb, np.float32)
    W_msg = np.asarray(W_msg, np.float32)
    W_upd = np.asarray(W_upd, np.float32)
    b_msg = np.asarray(b_msg, np.float32)
    b_upd = np.asarray(b_upd, np.float32)
    N = x.shape[0]

    meta, arr = _host_prep_diag(x, edge_index, edge_attr, x_agg_emb, W_msg, W_upd)
    NWIN = meta["NWIN"]
    with_bias = bool(np.any(b_msg) or np.any(b_upd))

    key = (N, NWIN, meta["T_A"], meta["T_B"], meta["NTA"], meta["NTB"], with_bias)
    if key not in _PROGRAM_CACHE:
        _PROGRAM_CACHE[key] = _build_program_diag(
            meta["NTA"], meta["NTB"], NWIN, meta["T_A"], meta["T_B"],
            meta["NTILES"], with_bias,
        )
    nc = _PROGRAM_CACHE[key]

    in_maps = []
    for c in range(C):
        m = dict(
            tabA=arr["tabA"],
            tabB=arr["tabB"],
            gidx=np.ascontiguousarray(arr["idx16"][c]),
            attrA=np.ascontiguousarray(arr["attrA"][c]),
            xT=np.ascontiguousarray(arr["xT"][c]),
            xaeT=np.ascontiguousarray(arr["xaeT"][c]),
            wm=arr["W_msg2"],
            wu=arr["W_upd2"],
        )
        if with_bias:
            m["bm"] = np.ascontiguousarray(np.tile(b_msg, (P, 1)))
            m["bu"] = np.ascontiguousarray(np.tile(b_upd, (P, 1)))
        in_maps.append(m)

    global LAST_RESULTS
    try:
        res = run_bass_kernel_spmd(nc, in_maps, core_ids=list(range(C)))
    except Exception:
        # device may be wedged from a prior run; reset via the axon
        # sidechannel and retry once.
        try:
            import ctypes

            lib = ctypes.CDLL("/opt/axon/libaxon_pjrt.so")
            lib.axon_reset.restype = ctypes.c_int64
            lib.axon_reset()
        except Exception:
            pass
        res = run_bass_kernel_spmd(nc, in_maps, core_ids=list(range(C)))
    LAST_RESULTS = res
    out_all = np.stack([r["out"] for r in res.results])  # [C, P, NWIN*64]

    node_pos = meta["node_pos"]
    result = out_all.reshape(C, P, NWIN, D)[
        meta["node_core"], node_pos % P, node_pos // P, :
    ]
    return np.ascontiguousarray(result.astype(np.float32))
